# revision 11
# baseline (speedup 1.0000x reference)
"""Trainium2 Bass kernel for nn_L1OutUB loss.

Math: the reference loss is
    result = mean_j positive[j] - mean_{i,j}(all_probs[i,j] + lse_mask - log(B-1))
Because only the MEAN over (i,j) is needed, the [B,B,D] pairwise block
collapses exactly (expanding the square) into rank-1 reductions:
    sum_{i,j,d} (y[j,d]-mu[i,d])^2 e[i,d]
      = sum_d Y2[d] E[d] - 2 sum_d Y1[d] ME[d] + B * C'
with e = exp(-lv), E = sum_i e, ME = sum_i mu*e, Y1 = sum_j y,
Y2 = sum_j y^2, C' = sum_{i,d} mu^2 e.  O(B D) instead of O(B^2 D).

Sharding: rows of x/y (dim 0) split across 8 cores (128 rows each).
Each core computes mu/logvar for its block via the two MLPs, the
elementwise stats, and reduces over its 128 rows via one ones-matmul
into a [1, 259] vector: [E | ME | Y1 | Y2 | C' | sum(lv) | sum((mu-y)^2 e)].
Host sums the 8 partials and evaluates the closed form in float64.

Written in raw Bass (no TileContext): this walrus build allows at most
one embedded sync-wait per instruction, so cross-engine deps use
standalone wait_ge instructions and monotonic per-engine semaphores.
Biases/ones are packed into the inputs host-side; biases are folded in
as rank-1 accumulating matmuls. Each PSUM tensor gets a full 2KB bank
so no engine reads a bank PE is concurrently writing.
"""

import sys

if "/opt/trn_rl_repo" not in sys.path:
    sys.path.insert(0, "/opt/trn_rl_repo")

from contextlib import ExitStack

import numpy as np

import concourse.bass as bass
from concourse import mybir
from concourse.bass_utils import run_bass_kernel_spmd

NCORES = 8
B = 1024
R = B // NCORES  # 128 rows per core
XD = 128         # x feature dim
H2 = 256         # hidden dim (HID // 2)
D = 64           # y feature dim
NCOLS = 4 * D + 3  # 259 output columns per core

# main input block [128, NIN] column layout
C_XT = 0      # x^T (k on partitions, i free)
C_W1M = 128
C_W2M = 384   # W2 packed [h, blk*64+d]
C_W1V = 512
C_W2V = 768
C_Y = 896
C_ONEK = 960  # column of ones (reduction lhsT)
NIN = 961
# row-strip input [1, NIN2]
C_B1M = 0
C_B1V = 256
C_B2M = 512
C_B2V = 576
C_ONER = 640  # row of 128 ones
NIN2 = 768

F32 = mybir.dt.float32
AF = mybir.ActivationFunctionType

TRACE = False
LAST_RESULTS = None

_NC = None


def _build():
    nc = bass.Bass(trn_type="TRN2")

    inp = nc.dram_tensor("inp", [128, NIN], F32, kind="ExternalInput")
    in2 = nc.dram_tensor("in2", [1, NIN2], F32, kind="ExternalInput")
    out = nc.dram_tensor("out", [1, NCOLS], F32, kind="ExternalOutput")

    with ExitStack() as ctx:
        e = ctx.enter_context
        IN = e(nc.sbuf_tensor([128, NIN], F32))
        IN2 = e(nc.sbuf_tensor([1, NIN2], F32))
        # one full 2KB PSUM bank per tensor (bank isolation)
        hm0 = e(nc.psum_tensor([128, 512], F32))
        hm1 = e(nc.psum_tensor([128, 512], F32))
        hv0 = e(nc.psum_tensor([128, 512], F32))
        hv1 = e(nc.psum_tensor([128, 512], F32))
        mu_b = e(nc.psum_tensor([128, 512], F32))
        lv_b = e(nc.psum_tensor([128, 512], F32))
        red_b = e(nc.psum_tensor([128, 512], F32))
        hm0_sb = e(nc.sbuf_tensor([128, R], F32))
        hm1_sb = e(nc.sbuf_tensor([128, R], F32))
        hv0_sb = e(nc.sbuf_tensor([128, R], F32))
        hv1_sb = e(nc.sbuf_tensor([128, R], F32))
        mu = e(nc.sbuf_tensor([R, D], F32))
        lv = e(nc.sbuf_tensor([R, D], F32))
        e_t = e(nc.sbuf_tensor([R, D], F32))
        dmy = e(nc.sbuf_tensor([R, D], F32))
        scr1 = e(nc.sbuf_tensor([R, D], F32))
        scr2 = e(nc.sbuf_tensor([R, D], F32))
        S = e(nc.sbuf_tensor([R, NCOLS], F32))
        out_sb = e(nc.sbuf_tensor([1, NCOLS], F32))
        dma_a = e(nc.semaphore())
        dma_b = e(nc.semaphore())
        dma_c = e(nc.semaphore())
        dma_d = e(nc.semaphore())
        dma_o = e(nc.semaphore())
        pe_s = e(nc.semaphore())
        act_s = e(nc.semaphore())
        dve_s = e(nc.semaphore())
        block = e(nc.Block())

        ones_r = IN2[0:1, C_ONER:C_ONER + 128]

        @block.sync
        def _(sync):
            # split input load across parallel DMA queues, ordered by consumer:
            # A = xT + W1m (+ the tiny row strip), B = W2m, C = W1v, D = rest
            sync.dma_start(out=IN2[:], in_=in2[:]).then_inc(dma_a, 16)
            sync.dma_start(
                out=IN[:, 0:C_W2M], in_=inp[:, 0:C_W2M]
            ).then_inc(dma_a, 16)
            sync.dma_start(
                out=IN[:, C_W2M:C_W1V], in_=inp[:, C_W2M:C_W1V]
            ).then_inc(dma_b, 16)
            sync.dma_start(
                out=IN[:, C_W1V:C_W2V], in_=inp[:, C_W1V:C_W2V]
            ).then_inc(dma_c, 16)
            sync.dma_start(
                out=IN[:, C_W2V:NIN], in_=inp[:, C_W2V:NIN]
            ).then_inc(dma_d, 16)
            sync.wait_ge(act_s, 8)
            sync.dma_start(out=out[:], in_=out_sb[:]).then_inc(dma_o, 16)
            sync.wait_ge(dma_o, 16)

        @block.tensor
        def _(tensor):
            tensor.wait_ge(dma_a, 32)
            # hidden pre-activations, bias folded as rank-1 matmul
            for h_ps, cw1, cb1, blk, wsem in (
                (hm0, C_W1M, C_B1M, 0, None), (hm1, C_W1M, C_B1M, 1, None),
                (hv0, C_W1V, C_B1V, 0, dma_c), (hv1, C_W1V, C_B1V, 1, None),
            ):
                if wsem is not None:
                    tensor.wait_ge(wsem, 16)
                tensor.matmul(
                    h_ps[:, 0:R],
                    IN[:, cw1 + blk * 128:cw1 + (blk + 1) * 128],
                    IN[:, C_XT:C_XT + R],
                    start=True, stop=False,
                )
                tensor.matmul(
                    h_ps[:, 0:R],
                    IN2[0:1, cb1 + blk * 128:cb1 + (blk + 1) * 128],
                    ones_r,
                    start=False, stop=True,
                ).then_inc(pe_s)
            # second layer + b2, for both branches
            for o_ps, h0, h1, cw2, cb2, wact, wsem in (
                (mu_b, hm0_sb, hm1_sb, C_W2M, C_B2M, 1, dma_b),
                (lv_b, hv0_sb, hv1_sb, C_W2V, C_B2V, 3, dma_d),
            ):
                tensor.wait_ge(wsem, 16)
                tensor.wait_ge(act_s, wact)
                tensor.matmul(
                    o_ps[:, 0:D], h0[:], IN[:, cw2:cw2 + D], start=True, stop=False
                )
                tensor.wait_ge(act_s, wact + 1)
                tensor.matmul(
                    o_ps[:, 0:D], h1[:], IN[:, cw2 + D:cw2 + 2 * D],
                    start=False, stop=False,
                )
                tensor.matmul(
                    o_ps[:, 0:D], ones_r, IN2[0:1, cb2:cb2 + D],
                    start=False, stop=True,
                ).then_inc(pe_s)
            # final partition reduction: column sums of S
            tensor.wait_ge(dve_s, 1)
            tensor.matmul(
                red_b[0:1, 0:NCOLS], IN[:, C_ONEK:C_ONEK + 1], S[:],
                start=True, stop=True,
            ).then_inc(pe_s)

        @block.scalar
        def _(scalar):
            for i, (h_sb, h_ps) in enumerate(
                ((hm0_sb, hm0), (hm1_sb, hm1), (hv0_sb, hv0), (hv1_sb, hv1))
            ):
                scalar.wait_ge(pe_s, i + 1)
                scalar.activation(
                    out=h_sb[:], in_=h_ps[:, 0:R], func=AF.Relu
                ).then_inc(act_s)  # act 1..4
            scalar.wait_ge(pe_s, 5)
            scalar.copy(out=mu[:], in_=mu_b[:, 0:D]).then_inc(act_s)      # act 5
            scalar.wait_ge(pe_s, 6)
            scalar.activation(
                out=lv[:], in_=lv_b[:, 0:D], func=AF.Tanh
            ).then_inc(act_s)                                             # act 6
            scalar.activation(
                out=e_t[:], in_=lv[:], func=AF.Exp, scale=-1.0
            ).then_inc(act_s)                                             # act 7
            scalar.wait_ge(pe_s, 7)
            scalar.copy(out=out_sb[:], in_=red_b[0:1, 0:NCOLS]).then_inc(act_s)  # 8

        @block.vector
        def _(vector):
            yv = IN[:, C_Y:C_Y + D]
            # S: [0:64]=e [64:128]=mu*e [128:192]=y [192:256]=y^2
            #    [256]=rowsum(mu^2 e) [257]=rowsum(lv) [258]=rowsum((mu-y)^2 e)
            vector.wait_ge(dma_d, 16)
            vector.tensor_copy(out=S[:, 2 * D:3 * D], in_=yv)
            vector.tensor_mul(out=S[:, 3 * D:4 * D], in0=yv, in1=yv)
            vector.wait_ge(act_s, 5)
            vector.tensor_sub(out=dmy[:], in0=mu[:], in1=yv)
            vector.tensor_mul(out=dmy[:], in0=dmy[:], in1=dmy[:])
            vector.wait_ge(act_s, 7)
            vector.tensor_copy(out=S[:, 0:D], in_=e_t[:])
            vector.tensor_mul(out=S[:, D:2 * D], in0=mu[:], in1=e_t[:])
            vector.tensor_mul(out=scr1[:], in0=mu[:], in1=S[:, D:2 * D])
            vector.reduce_sum(
                out=S[:, 4 * D:4 * D + 1], in_=scr1[:], axis=mybir.AxisListType.X
            )
            vector.reduce_sum(
                out=S[:, 4 * D + 1:4 * D + 2], in_=lv[:], axis=mybir.AxisListType.X
            )
            vector.tensor_mul(out=scr2[:], in0=dmy[:], in1=e_t[:])
            vector.reduce_sum(
                out=S[:, 4 * D + 2:4 * D + 3], in_=scr2[:], axis=mybir.AxisListType.X
            ).then_inc(dve_s)

    return nc


def kernel(x_samples, y_samples, Wm1, bm1, Wm2, bm2, Wv1, bv1, Wv2, bv2):
    global _NC, LAST_RESULTS
    if _NC is None:
        _NC = _build()

    f = lambda a: np.asarray(a, dtype=np.float32)
    x, y = f(x_samples), f(y_samples)

    w2m = f(Wm2).reshape(2, 128, D).transpose(1, 0, 2).reshape(128, 2 * D)
    w2v = f(Wv2).reshape(2, 128, D).transpose(1, 0, 2).reshape(128, 2 * D)
    in2 = np.zeros((1, NIN2), dtype=np.float32)
    in2[0, C_B1M:C_B1M + H2] = f(bm1)
    in2[0, C_B1V:C_B1V + H2] = f(bv1)
    in2[0, C_B2M:C_B2M + D] = f(bm2)
    in2[0, C_B2V:C_B2V + D] = f(bv2)
    in2[0, C_ONER:C_ONER + 128] = 1.0

    in_maps = []
    for c in range(NCORES):
        blk = slice(c * R, (c + 1) * R)
        inp = np.empty((128, NIN), dtype=np.float32)
        inp[:, C_XT:C_XT + R] = x[blk].T
        inp[:, C_W1M:C_W1M + H2] = f(Wm1)
        inp[:, C_W2M:C_W2M + 2 * D] = w2m
        inp[:, C_W1V:C_W1V + H2] = f(Wv1)
        inp[:, C_W2V:C_W2V + 2 * D] = w2v
        inp[:, C_Y:C_Y + D] = y[blk]
        inp[:, C_ONEK] = 1.0
        in_maps.append({"inp": inp, "in2": in2})

    res = run_bass_kernel_spmd(_NC, in_maps, core_ids=list(range(NCORES)), trace=TRACE)
    LAST_RESULTS = res

    s = np.stack([r["out"][0] for r in res.results]).astype(np.float64).sum(axis=0)
    E, ME = s[0:D], s[D:2 * D]
    Y1, Y2 = s[2 * D:3 * D], s[3 * D:4 * D]
    Cp, Ls, P1 = s[4 * D], s[4 * D + 1], s[4 * D + 2]

    T = float(np.dot(Y2, E) - 2.0 * np.dot(Y1, ME)) + B * Cp
    sum_all_probs = -0.5 * T - 0.5 * B * Ls
    P = -0.5 * P1 - 0.5 * Ls
    lse_mask = np.log((B - 1.0) + np.exp(-20.0))
    result = P / B - sum_all_probs / (B * B) - lse_mask + np.log(B - 1.0)
    return np.asarray(result, dtype=np.float32)
